# revision 14
# baseline (speedup 1.0000x reference)
"""Trainium2 Bass kernel for nn_BoundaryLoss (3D-Laplacian boundary loss).

reference semantics (fp32):
    probs = softmax(logits, axis=1)[:, 1:]                  # (B, C-1, D, H, W)
    tmask = one_hot(targets)[classes 1..C-1]                # (B, C-1, D, H, W)
    loss  = mean((|lap3(probs)| - |lap3(tmask)|)**2)        # lap3 = 6-neighbour
                                                            # Laplacian, zero pad

Key identity: with a = lap(p), b = lap(m),
    (|a| - |b|)^2 = min((a-b)^2, (a+b)^2) = min(lap(p-m)^2, lap(p+m)^2)
so each (class, voxel) needs two PE stencils c = lap(p)-lap(m),
s = lap(p)+lap(m) (accumulated in PSUM) and a small drain:
uv = {c,s}^2, slot += sum(min(u, v)).

Distribution: pure data parallelism over H (256 rows -> 8 slices of 32 rows,
one halo row each side).  Host marshals fp8 logits (pad sentinel: class0 -> 0,
classes 1..3 -> NEG so softmax probs vanish at pads) and fp8 one-hot masks,
both padded one column left/right so the w+-1 taps are shifted full-width
reads.

On-core layout: partitions = (d, b) interleaved p = 2*d + b (T_D tridiagonal
stationary covers d+-1 and the -6 centre).  Laplacians run as fp8 DoubleRow
matmuls: one instruction contracts TWO k-tiles - the p-plane tap and the
m-plane tap of the U tile; +-weight pairs select lap(p)-+lap(m) - at 0.5
cycles/output element (4x the bf16 rate).  5 taps per output row: T_D, h-1,
h+1 (row +-1), w-1, w+1 (col +-1 into the pad columns).

Softmax: ScalarE exp (bias -2 keeps fp8 e < 32), denominator S via fp8
DoubleRow identity accumulation over class pairs, DVE fast reciprocal,
p = e*r on DVE.

PSUM (8 banks x 2KB = 16 rows of 256 f32): banks 0-1 = S chunks (4 rows,
single-buffered), banks 2-7 = three {c(2 rows), s(2 rows)} lap buffers
rotating.  PSUM accumulation groups are bank-aligned (start=True zeroes the
2KB bank).

Drain identity: sum(min(u,v)) = sum(u+v) - sum(max(u,v)), accumulated as
slotP (plus side) and slotM (max side); host computes (P - M)/N.  GPSIMD
cannot read PSUM, and any one instruction may read PSUM through only one
operand, so:
  late sets (t >= E_SPLIT, after exp frees the ScalarE): ScalarE Square
    (one PSUM read) -> uv=(c^2,s^2) with free accum_out = slotP; then DVE
    tensor_tensor max + reduce_sum -> slotM.
  early sets (exp still running): DVE tensor_copy (frees PSUM promptly),
    DVE square in SBUF, then lag-tolerant reduces -> slotP/slotM, flushed
    at block boundaries.
(tensor_tensor_reduce faults the HW - replaced with TT + reduce_sum.)
p = e*r runs as GpSimd tensor_tensor for 6 (chunk, class) pairs and DVE for
the rest; reciprocal on DVE.  PE: warmup, S, 960 DoubleRow lap matmuls.
"""

import numpy as np
import ml_dtypes

import concourse.bass as bass
import concourse.bacc as bacc
import concourse.tile as tile
from concourse import mybir
from concourse.bass_utils import run_bass_kernel_spmd

# Problem shape (hardcoded; harness contract)
B, C, D, H, W = 2, 4, 64, 256, 256
NCORES = 8
HS = H // NCORES        # 32 output rows per core
HL = HS + 2             # 34 input rows (1 halo row each side)
WP = W + 2              # 258: one zero-prob pad column each side
NEG = -100.0            # pad logit for classes 1..3 -> prob ~ 0
EXP_BIAS = -2.0         # e = exp(l - 2): keeps fp8 e well under the fp8 max
NTOT = B * (C - 1) * D * H * W  # mean denominator

GROUP = 2               # output rows per lap psum bank
NGRP = HS // GROUP      # 16 groups per class pair
NT = NGRP * 3           # 48 (group, pair) drain sets -> accumulator slots
E_SPLIT = 6             # sets < split drain via the DVE copy path (ScalarE
                        # is still busy with exp); later sets via ScalarE

F32 = mybir.dt.float32
BF16 = mybir.dt.bfloat16
F8 = mybir.dt.float8e4
F8ML = ml_dtypes.float8_e4m3
AX = mybir.AxisListType
OP = mybir.AluOpType
AF = mybir.ActivationFunctionType
DR = mybir.MatmulPerfMode.DoubleRow

N_WARMUP = 32  # junk matmuls to open the PE clock gate early

# 8-row chunks over HL rows: logits DMA / exp / p-mult granularity
CHUNKS = [(r0, min(8, HL - r0)) for r0 in range(0, HL, 8)]
# 4-row subchunks: softmax-denominator + reciprocal granularity
SCHUNKS = [(q0, min(4, HL - q0)) for q0 in range(0, HL, 4)]


def _stationaries():
    """T_D: d-stencil (d+-1 within the same b, -6 centre) on the interleaved
    partition layout p = 2*d + b, and the identity.  Exact in fp8."""
    td = np.zeros((128, 128), dtype=np.float32)
    for p in range(128):
        td[p, p] = -6.0
        d, _ = divmod(p, 2)
        if d > 0:
            td[p - 2, p] = 1.0
        if d < D - 1:
            td[p + 2, p] = 1.0
    ident = np.eye(128, dtype=np.float32)
    return td, ident


def _emit(tc):
    nc = tc.nc
    lg = nc.dram_tensor("logits", [C, 128, HL, WP], F8, kind="ExternalInput").ap()
    mh = nc.dram_tensor("masks", [128, 3, HL, WP], F8, kind="ExternalInput").ap()
    wt = nc.dram_tensor("wts", [128, 4, 2, 128], F8, kind="ExternalInput").ap()
    wid = nc.dram_tensor("wI", [128, 128], F8, kind="ExternalInput").ap()
    out_d = nc.dram_tensor("out", [128, 2], F32, kind="ExternalOutput").ap()

    with (
        tc.tile_pool(name="singles", bufs=1) as singles,
        tc.tile_pool(name="uvpool", bufs=14) as uvpool,
        tc.tile_pool(name="psum", bufs=1, space="PSUM") as psum,
    ):
        # --- persistent tiles ---
        xl = singles.tile([128, C, HL, WP], F8, tag="xl")     # logits -> e
        # U[:,0,c] = p_{c+1} (DVE-written), U[:,1,c] = m_{c+1} (DMA'd)
        U = singles.tile([128, 2, 3, HL, WP], F8, tag="U")
        rf = singles.tile([128, HL, WP], F32, tag="rf")       # 1/S (pads = 1)
        w4 = singles.tile([128, 4, 2, 128], F8, tag="w4")     # DR weight pairs
        wi = singles.tile([128, 128], F8, tag="wi")           # warmup identity
        slots = singles.tile([128, 2 * NT], F32, tag="slots")
        res = singles.tile([128, 2], F32, tag="res")
        scrd = singles.tile([128, 2 * GROUP, W], BF16, tag="scrd")  # DVE scr
        scrg = singles.tile([128, GROUP, W], BF16, tag="scrg")  # Gp scr
        ebias = singles.tile([128, 1], F32, tag="ebias")      # exp bias const
        arena = psum.tile([128, 16, W], F32, tag="arena")     # full PSUM

        # --- DMA staging ---
        nc.sync.dma_start(out=w4, in_=wt)
        nc.sync.dma_start(out=wi, in_=wid)
        # masks in row chunks so early lap groups don't wait the full 3.4 MB
        for r0, nr in CHUNKS:
            nc.sync.dma_start(out=U[:, 1, :, r0:r0 + nr, :],
                              in_=mh[:, :, r0:r0 + nr, :])
        for r0, nr in CHUNKS:
            for ci in range(C):
                nc.gpsimd.dma_start(out=xl[:, ci, r0:r0 + nr, :],
                                    in_=lg[ci, :, r0:r0 + nr, :])

        # rf pad columns: p = e*r needs finite r there (e = 0 -> p = 0)
        nc.vector.memset(rf[:, :, 0:1], 1.0)
        nc.vector.memset(rf[:, :, WP - 1:WP], 1.0)
        nc.vector.memset(ebias, EXP_BIAS)

        # --- PE warmup: ramp the clock gate before real matmuls ---
        for i in range(N_WARMUP):
            nc.tensor.matmul(out=arena[:, 0:1, 0:128], lhsT=wi,
                             rhs=wi[:, 0:128],
                             start=(i == 0), stop=(i == N_WARMUP - 1))

        # --- ScalarE: exp over all 4 classes per chunk, in place ---
        for r0, nr in CHUNKS:
            nc.scalar.activation(xl[:, :, r0:r0 + nr, :],
                                 xl[:, :, r0:r0 + nr, :], AF.Exp,
                                 bias=ebias)

        def denom(k):
            """S = sum_c e_c for subchunk k -> arena rows 0..qr-1 (banks 0-1),
            then rf rows = 1/S.  fp8 DoubleRow over class pairs."""
            q0, qr = SCHUNKS[k]
            for j in range(qr):
                for cc in range(2):
                    nc.tensor.matmul(
                        out=arena[:, j:j + 1, 0:W], lhsT=w4[:, 3],
                        rhs=xl[:, 2 * cc:2 * cc + 2, q0 + j, 1:1 + W],
                        start=(j % 2 == 0 and cc == 0),
                        stop=(j % 2 == 1 and cc == 1),
                        perf_mode=DR)
            nc.vector.reciprocal_approx_fast(
                out=rf[:, q0:q0 + qr, 1:1 + W], in_=arena[:, 0:qr, 0:W])

        def pmul(k):
            """p = e * r for chunk k, classes 1..3 -> U[:, 0].  GpSimd STT
            (efficiency 0.6) for two classes keeps the DVE free for drains
            during the exp phase; DVE tensor_tensor for the third."""
            r0, nr = CHUNKS[k]
            for ci in range(3):
                eng = nc.gpsimd if (ci == 2 and k < 4) or (ci == 1 and k < 2) \
                    else nc.vector
                eng.tensor_tensor(
                    out=U[:, 0, ci, r0:r0 + nr, :],
                    in0=xl[:, ci + 1, r0:r0 + nr, :],
                    in1=rf[:, r0:r0 + nr, :], op=OP.mult)

        # lap taps: (row offset, col start, weight idx for c, for s)
        # w4[:,0]=[T_D,-T_D] w4[:,1]=[I,-I] w4[:,2]=[T_D,T_D] w4[:,3]=[I,I]
        TAPS = [(0, 1, 0, 2), (-1, 1, 1, 3), (1, 1, 1, 3),
                (0, 0, 1, 3), (0, 2, 1, 3)]

        tctr = [0]
        pend = []  # early sets' deferred TTR reduces: (t, sq tile)

        def lapset(g, pi):
            """c/s = lap(p)-+lap(m) for pair pi, out rows 2g..2g+1, then
            uv = {c,s}^2 and slots[t] = sum(min(u, v))."""
            t = tctr[0]
            tctr[0] += 1
            cr = 4 + 4 * (t % 3)
            for base, wsel in ((cr, 2), (cr + 2, 3)):
                for j in range(GROUP):
                    rj = 1 + GROUP * g + j
                    for ti, (dr_, c0, wc, ws) in enumerate(TAPS):
                        nc.tensor.matmul(
                            out=arena[:, base + j:base + j + 1, 0:W],
                            lhsT=w4[:, wc if wsel == 2 else ws],
                            rhs=U[:, 0:2, pi, rj + dr_, c0:c0 + W],
                            start=(j == 0 and ti == 0),
                            stop=(j == GROUP - 1 and ti == len(TAPS) - 1),
                            perf_mode=DR)
            uv = uvpool.tile([128, 2 * GROUP, W], BF16, tag="uv")
            if t < E_SPLIT:
                # ScalarE is exp-busy: copy PSUM off promptly on the DVE,
                # square in SBUF, defer the two reduces
                nc.vector.tensor_copy(uv, arena[:, cr:cr + 4, 0:W])
                sq = uvpool.tile([128, 2 * GROUP, W], BF16, tag="sq")
                nc.vector.tensor_tensor(out=sq, in0=uv, in1=uv, op=OP.mult)
                pend.append((t, sq))
            else:
                # one PSUM read; accum_out = sum(c^2 + s^2) = slotP for free
                nc.scalar.activation(uv, arena[:, cr:cr + 4, 0:W], AF.Square,
                                     accum_out=slots[:, t:t + 1])
                nc.vector.tensor_tensor(
                    out=scrd[:, 0:GROUP], in0=uv[:, 0:GROUP],
                    in1=uv[:, GROUP:2 * GROUP], op=OP.max)
                nc.vector.reduce_sum(
                    out=slots[:, NT + t:NT + t + 1],
                    in_=scrd[:, 0:GROUP], axis=AX.XY)

        def flush_pend():
            for t, sq in pend:
                nc.vector.reduce_sum(
                    out=slots[:, t:t + 1], in_=sq, axis=AX.XY)
                nc.vector.tensor_tensor(
                    out=scrd[:, GROUP:2 * GROUP], in0=sq[:, 0:GROUP],
                    in1=sq[:, GROUP:2 * GROUP], op=OP.max)
                nc.vector.reduce_sum(
                    out=slots[:, NT + t:NT + t + 1],
                    in_=scrd[:, GROUP:2 * GROUP], axis=AX.XY)
            pend.clear()

        # --- interleaved emission in dependency order ---
        # lap group g needs p/m rows <= 2g+2; p chunk k covers rows < 8(k+1)
        denom(0)
        denom(1)
        pmul(0)
        for g in range(3):          # rows <= 7
            for pi in range(3):
                lapset(g, pi)
        denom(2)
        denom(3)
        pmul(1)
        flush_pend()
        for g in range(3, 7):       # rows <= 15
            for pi in range(3):
                lapset(g, pi)
        denom(4)
        denom(5)
        pmul(2)
        flush_pend()
        for g in range(7, 11):      # rows <= 23
            for pi in range(3):
                lapset(g, pi)
        denom(6)
        denom(7)
        pmul(3)
        for g in range(11, 15):     # rows <= 31
            for pi in range(3):
                lapset(g, pi)
        denom(8)
        pmul(4)
        for pi in range(3):         # rows <= 33
            lapset(15, pi)
        flush_pend()

        nc.vector.reduce_sum(out=res[:, 0:1], in_=slots[:, 0:NT], axis=AX.X)
        nc.vector.reduce_sum(out=res[:, 1:2], in_=slots[:, NT:2 * NT],
                             axis=AX.X)
        nc.sync.dma_start(out=out_d, in_=res)


def build_nc():
    nc = bacc.Bacc("TRN2", target_bir_lowering=False, debug=False)
    with tile.TileContext(nc) as tc:
        _emit(tc)
    nc.compile()
    return nc


_CACHE = {}


def _get_nc():
    if "nc" not in _CACHE:
        _CACHE["nc"] = build_nc()
    return _CACHE["nc"]


def make_in_maps(logits, targets):
    """Host-side marshaling: pad H/W, one-hot, interleave partitions,
    fp8-cast, slice per core."""
    logits = np.asarray(logits, dtype=np.float32)
    targets = np.asarray(targets)
    Hp, Wp = H + 2, W + 2

    # logits: pad rows+cols with (class0 -> 0, classes 1..3 -> NEG) so the
    # softmax probs vanish at pads, matching the reference's zero-padded conv
    lp = np.zeros((B, C, D, Hp, Wp), np.float32)
    lp[:, 1:] = NEG
    lp[:, :, :, 1:H + 1, 1:W + 1] = logits
    # partitions p = 2*d + b
    lp = lp.transpose(1, 2, 0, 3, 4).reshape(C, 2 * D, Hp, Wp).astype(F8ML)

    # one-hot masks for classes 1..3, zero pads; layout [p, cls, h, w]
    mk = np.zeros((3, B, D, Hp, Wp), np.float32)
    for i, c in enumerate((1, 2, 3)):
        mk[i, :, :, 1:H + 1, 1:W + 1] = (targets == c)
    mk = mk.transpose(2, 1, 0, 3, 4).reshape(2 * D, 3, Hp, Wp).astype(F8ML)

    td, ident = _stationaries()
    w4 = np.empty((128, 4, 2, 128), np.float32)
    w4[:, 0, 0], w4[:, 0, 1] = td, -td         # c: T_D tap
    w4[:, 1, 0], w4[:, 1, 1] = ident, -ident   # c: shift taps
    w4[:, 2, 0], w4[:, 2, 1] = td, td          # s: T_D tap
    w4[:, 3, 0], w4[:, 3, 1] = ident, ident    # s: shift taps / denominator
    w4 = w4.astype(F8ML)
    wi = ident.astype(F8ML)

    in_maps = []
    for k in range(NCORES):
        h0 = k * HS
        in_maps.append({
            "logits": np.ascontiguousarray(lp[:, :, h0:h0 + HL, :]),
            "masks": np.ascontiguousarray(mk[:, :, h0:h0 + HL, :]),
            "wts": w4,
            "wI": wi,
        })
    return in_maps


def kernel(logits, targets):
    nc = _get_nc()
    in_maps = make_in_maps(logits, targets)
    results = run_bass_kernel_spmd(nc, in_maps, core_ids=list(range(NCORES)))
    total = 0.0
    for r in results.results:
        o = np.asarray(r["out"], dtype=np.float64)
        total += o[:, 0].sum() - o[:, 1].sum()
    return np.float32(total / NTOT)


# revision 17
# speedup vs baseline: 1.1236x; 1.1236x over previous
"""Trainium2 Bass kernel for nn_BoundaryLoss (3D-Laplacian boundary loss).

reference semantics (fp32):
    probs = softmax(logits, axis=1)[:, 1:]                  # (B, C-1, D, H, W)
    tmask = one_hot(targets)[classes 1..C-1]                # (B, C-1, D, H, W)
    loss  = mean((|lap3(probs)| - |lap3(tmask)|)**2)        # lap3 = 6-neighbour
                                                            # Laplacian, zero pad

Key identity: with a = lap(p), b = lap(m),
    (|a| - |b|)^2 = min((a-b)^2, (a+b)^2) = min(lap(p-m)^2, lap(p+m)^2)
so each (class, voxel) needs two PE stencils c = lap(p)-lap(m),
s = lap(p)+lap(m) (accumulated in PSUM) and a small drain:
uv = {c,s}^2, slot += sum(min(u, v)).

Distribution: pure data parallelism over H (256 rows -> 8 slices of 32 rows,
one halo row each side).  Host marshals fp8 logits (pad sentinel: class0 -> 0,
classes 1..3 -> NEG so softmax probs vanish at pads) and fp8 one-hot masks,
both padded one column left/right so the w+-1 taps are shifted full-width
reads.

On-core layout: partitions = (d, b) interleaved p = 2*d + b (T_D tridiagonal
stationary covers d+-1 and the -6 centre).  Laplacians run as fp8 DoubleRow
matmuls: one instruction contracts TWO k-tiles - the p-plane tap and the
m-plane tap of the U tile; +-weight pairs select lap(p)-+lap(m) - at 0.5
cycles/output element (4x the bf16 rate).  5 taps per output row: T_D, h-1,
h+1 (row +-1), w-1, w+1 (col +-1 into the pad columns).

Softmax: ScalarE exp (bias -2 keeps fp8 e < 32), denominator S via fp8
DoubleRow identity accumulation over class pairs, DVE fast reciprocal,
p = e*r on DVE.

PSUM (8 banks x 2KB = 16 rows of 256 f32): banks 0-1 = S chunks (4 rows,
single-buffered), banks 2-7 = three {c(2 rows), s(2 rows)} lap buffers
rotating.  PSUM accumulation groups are bank-aligned (start=True zeroes the
2KB bank).

Drain identity: sum(min(u,v)) = sum(u+v) - sum(max(u,v)), accumulated as
slotP (plus side) and slotM (max side); host computes (P - M)/N.  GPSIMD
cannot read PSUM, and any one instruction may read PSUM through only one
operand, so:
  late sets (t >= E_SPLIT, after exp frees the ScalarE): ScalarE Square
    (one PSUM read) -> uv=(c^2,s^2) with free accum_out = slotP; then DVE
    tensor_tensor max + reduce_sum -> slotM.
  early sets (exp still running): DVE tensor_copy (frees PSUM promptly),
    DVE square in SBUF, then lag-tolerant reduces -> slotP/slotM, flushed
    at block boundaries.
(tensor_tensor_reduce faults the HW - replaced with TT + reduce_sum.)
p = e*r runs as GpSimd tensor_tensor for 6 (chunk, class) pairs and DVE for
the rest; reciprocal on DVE.  PE: warmup, S, 960 DoubleRow lap matmuls.
"""

import numpy as np
import ml_dtypes

import concourse.bass as bass
import concourse.bacc as bacc
import concourse.tile as tile
from concourse import mybir
from concourse.bass_utils import run_bass_kernel_spmd

# Problem shape (hardcoded; harness contract)
B, C, D, H, W = 2, 4, 64, 256, 256
NCORES = 8
HS = H // NCORES        # 32 output rows per core
HL = HS + 2             # 34 input rows (1 halo row each side)
WP = W + 2              # 258: one zero-prob pad column each side
NEG = -100.0            # pad logit for classes 1..3 -> prob ~ 0
EXP_BIAS = -2.0         # e = exp(l - 2): keeps fp8 e well under the fp8 max
NTOT = B * (C - 1) * D * H * W  # mean denominator

GROUP = 2               # output rows per lap psum bank
NGRP = HS // GROUP      # 16 groups per class pair
NT = NGRP * 3           # 48 (group, pair) drain sets -> accumulator slots
E_SPLIT = 9             # sets < split drain via the DVE copy path (ScalarE
                        # is still busy with exp); later sets via ScalarE

F32 = mybir.dt.float32
BF16 = mybir.dt.bfloat16
F8 = mybir.dt.float8e4
F8ML = ml_dtypes.float8_e4m3
AX = mybir.AxisListType
OP = mybir.AluOpType
AF = mybir.ActivationFunctionType
DR = mybir.MatmulPerfMode.DoubleRow

N_WARMUP = 32  # junk matmuls to open the PE clock gate early

# 8-row chunks over HL rows: logits DMA / exp / p-mult granularity
CHUNKS = [(r0, min(8, HL - r0)) for r0 in range(0, HL, 8)]
# 4-row subchunks: softmax-denominator + reciprocal granularity
SCHUNKS = [(q0, min(4, HL - q0)) for q0 in range(0, HL, 4)]


def _stationaries():
    """T_D: d-stencil (d+-1 within the same b, -6 centre) on the interleaved
    partition layout p = 2*d + b, and the identity.  Exact in fp8."""
    td = np.zeros((128, 128), dtype=np.float32)
    for p in range(128):
        td[p, p] = -6.0
        d, _ = divmod(p, 2)
        if d > 0:
            td[p - 2, p] = 1.0
        if d < D - 1:
            td[p + 2, p] = 1.0
    ident = np.eye(128, dtype=np.float32)
    return td, ident


def _emit(tc):
    nc = tc.nc
    lg = nc.dram_tensor("logits", [128, C, HL, WP], F8, kind="ExternalInput").ap()
    mh = nc.dram_tensor("masks", [128, 3, HL, WP], F8, kind="ExternalInput").ap()
    wt = nc.dram_tensor("wts", [128, 4, 2, 128], F8, kind="ExternalInput").ap()
    wid = nc.dram_tensor("wI", [128, 128], F8, kind="ExternalInput").ap()
    out_d = nc.dram_tensor("out", [128, 2], F32, kind="ExternalOutput").ap()

    with (
        tc.tile_pool(name="singles", bufs=1) as singles,
        tc.tile_pool(name="uvpool", bufs=14) as uvpool,
        tc.tile_pool(name="psum", bufs=1, space="PSUM") as psum,
    ):
        # --- persistent tiles ---
        xl = singles.tile([128, C, HL, WP], F8, tag="xl")     # logits -> e
        # U[:,0,c] = p_{c+1} (DVE-written), U[:,1,c] = m_{c+1} (DMA'd)
        U = singles.tile([128, 2, 3, HL, WP], F8, tag="U")
        rf = singles.tile([128, HL, WP], F32, tag="rf")       # 1/S (pads = 1)
        w4 = singles.tile([128, 4, 2, 128], F8, tag="w4")     # DR weight pairs
        wi = singles.tile([128, 128], F8, tag="wi")           # warmup identity
        slots = singles.tile([128, 2 * NT], F32, tag="slots")
        res = singles.tile([128, 2], F32, tag="res")
        scrd = singles.tile([128, 2 * GROUP, W], BF16, tag="scrd")  # DVE scr
        macc = singles.tile([128, 8, GROUP, W], BF16, tag="macc")  # max batch
        ebias = singles.tile([128, 1], F32, tag="ebias")      # exp bias const
        arena = psum.tile([128, 16, W], F32, tag="arena")     # full PSUM

        # --- DMA staging ---
        nc.sync.dma_start(out=w4, in_=wt)
        nc.sync.dma_start(out=wi, in_=wid)
        # one DMA per chunk (few descriptors, ~1us SWDGE each); masks ride
        # the same queue after each logits chunk so chunk 0 lands first
        for r0, nr in CHUNKS:
            nc.gpsimd.dma_start(out=xl[:, :, r0:r0 + nr, :],
                                in_=lg[:, :, r0:r0 + nr, :])
            nc.gpsimd.dma_start(out=U[:, 1, :, r0:r0 + nr, :],
                                in_=mh[:, :, r0:r0 + nr, :])

        # rf pad columns: p = e*r needs finite r there (e = 0 -> p = 0)
        nc.vector.memset(rf[:, :, 0:1], 1.0)
        nc.vector.memset(rf[:, :, WP - 1:WP], 1.0)
        nc.vector.memset(ebias, EXP_BIAS)

        # --- PE warmup: ramp the clock gate before real matmuls ---
        for i in range(N_WARMUP):
            nc.tensor.matmul(out=arena[:, 0:1, 0:128], lhsT=wi,
                             rhs=wi[:, 0:128],
                             start=(i == 0), stop=(i == N_WARMUP - 1))

        # --- ScalarE: exp over all 4 classes per chunk, in place ---
        for r0, nr in CHUNKS:
            nc.scalar.activation(xl[:, :, r0:r0 + nr, :],
                                 xl[:, :, r0:r0 + nr, :], AF.Exp,
                                 bias=ebias)

        def denom(k):
            """S = sum_c e_c for subchunk k -> arena rows 0..qr-1 (banks 0-1),
            then rf rows = 1/S.  fp8 DoubleRow over class pairs."""
            q0, qr = SCHUNKS[k]
            for j in range(0, qr, 2):
                for cc in range(2):
                    nc.tensor.matmul(
                        out=arena[:, j:j + 2, 0:W], lhsT=w4[:, 3],
                        rhs=xl[:, 2 * cc:2 * cc + 2, q0 + j:q0 + j + 2,
                               1:1 + W],
                        start=(cc == 0), stop=(cc == 1),
                        perf_mode=DR)
            nc.vector.reciprocal_approx_fast(
                out=rf[:, q0:q0 + qr, 1:1 + W], in_=arena[:, 0:qr, 0:W])

        def pmul(k):
            """p = e * r for chunk k, classes 1..3 -> U[:, 0].  GpSimd STT
            (efficiency 0.6) for two classes keeps the DVE free for drains
            during the exp phase; DVE tensor_tensor for the third."""
            r0, nr = CHUNKS[k]
            for ci in range(3):
                eng = nc.gpsimd if (ci == 2) or (ci == 1 and k < 2) \
                    else nc.vector
                eng.tensor_tensor(
                    out=U[:, 0, ci, r0:r0 + nr, :],
                    in0=xl[:, ci + 1, r0:r0 + nr, :],
                    in1=rf[:, r0:r0 + nr, :], op=OP.mult)

        # lap taps: (row offset, col start, weight idx for c, for s)
        # w4[:,0]=[T_D,-T_D] w4[:,1]=[I,-I] w4[:,2]=[T_D,T_D] w4[:,3]=[I,I]
        TAPS = [(0, 1, 0, 2), (-1, 1, 1, 3), (1, 1, 1, 3),
                (0, 0, 1, 3), (0, 2, 1, 3)]

        tctr = [0]
        pend = []  # early sets' deferred TTR reduces: (t, sq tile)

        def lapset(g, pi):
            """c/s = lap(p)-+lap(m) for pair pi, out rows 2g..2g+1, then
            uv = {c,s}^2 and slots[t] = sum(min(u, v))."""
            t = tctr[0]
            tctr[0] += 1
            cr = 4 + 4 * (t % 3)
            rj = 1 + GROUP * g
            for base, wsel in ((cr, 2), (cr + 2, 3)):
                for ti, (dr_, c0, wc, ws) in enumerate(TAPS):
                    nc.tensor.matmul(
                        out=arena[:, base:base + GROUP, 0:W],
                        lhsT=w4[:, wc if wsel == 2 else ws],
                        rhs=U[:, 0:2, pi, rj + dr_:rj + dr_ + GROUP,
                              c0:c0 + W],
                        start=(ti == 0), stop=(ti == len(TAPS) - 1),
                        perf_mode=DR)
            uv = uvpool.tile([128, 2 * GROUP, W], BF16, tag="uv")
            if t < E_SPLIT:
                # ScalarE is exp-busy: copy PSUM off promptly on the DVE,
                # square in SBUF, defer the two reduces
                nc.vector.tensor_copy(uv, arena[:, cr:cr + 4, 0:W])
                sq = uvpool.tile([128, 2 * GROUP, W], BF16, tag="sq")
                nc.vector.tensor_tensor(out=sq, in0=uv, in1=uv, op=OP.mult)
                pend.append((t, sq))
            else:
                # one PSUM read; accum_out = sum(c^2 + s^2) = slotP for free
                nc.scalar.activation(uv, arena[:, cr:cr + 4, 0:W], AF.Square,
                                     accum_out=slots[:, t:t + 1])
                nc.vector.tensor_tensor(
                    out=macc[:, t % 8], in0=uv[:, 0:GROUP],
                    in1=uv[:, GROUP:2 * GROUP], op=OP.max)
                if t % 8 == 7 or t == NT - 1:
                    nc.vector.reduce_sum(
                        out=slots[:, NT + t // 8:NT + t // 8 + 1],
                        in_=macc[:, 0:(t % 8) + 1], axis=AX.XYZ)

        def flush_pend():
            for t, sq in pend:
                nc.vector.reduce_sum(
                    out=slots[:, t:t + 1], in_=sq, axis=AX.XY)
                nc.vector.tensor_tensor(
                    out=macc[:, t % 8], in0=sq[:, 0:GROUP],
                    in1=sq[:, GROUP:2 * GROUP], op=OP.max)
                if t % 8 == 7:
                    nc.vector.reduce_sum(
                        out=slots[:, NT + t // 8:NT + t // 8 + 1],
                        in_=macc, axis=AX.XYZ)
            pend.clear()

        # --- interleaved emission in dependency order ---
        # lap group g needs p/m rows <= 2g+2; p chunk k covers rows < 8(k+1)
        denom(0)
        denom(1)
        pmul(0)
        for g in range(3):          # rows <= 7
            for pi in range(3):
                lapset(g, pi)
        denom(2)
        denom(3)
        pmul(1)
        flush_pend()
        for g in range(3, 7):       # rows <= 15
            for pi in range(3):
                lapset(g, pi)
        denom(4)
        denom(5)
        pmul(2)
        flush_pend()
        for g in range(7, 11):      # rows <= 23
            for pi in range(3):
                lapset(g, pi)
        denom(6)
        denom(7)
        pmul(3)
        for g in range(11, 15):     # rows <= 31
            for pi in range(3):
                lapset(g, pi)
        denom(8)
        pmul(4)
        for pi in range(3):         # rows <= 33
            lapset(15, pi)
        flush_pend()

        nc.vector.reduce_sum(out=res[:, 0:1], in_=slots[:, 0:NT], axis=AX.X)
        nc.vector.reduce_sum(out=res[:, 1:2], in_=slots[:, NT:NT + 6],
                             axis=AX.X)
        nc.sync.dma_start(out=out_d, in_=res)


def build_nc():
    nc = bacc.Bacc("TRN2", target_bir_lowering=False, debug=False)
    with tile.TileContext(nc) as tc:
        _emit(tc)
    nc.compile()
    return nc


_CACHE = {}


def _get_nc():
    if "nc" not in _CACHE:
        _CACHE["nc"] = build_nc()
    return _CACHE["nc"]


def make_in_maps(logits, targets):
    """Host-side marshaling: pad H/W, one-hot, interleave partitions,
    fp8-cast, slice per core."""
    logits = np.asarray(logits, dtype=np.float32)
    targets = np.asarray(targets)
    Hp, Wp = H + 2, W + 2

    # logits: pad rows+cols with (class0 -> 0, classes 1..3 -> NEG) so the
    # softmax probs vanish at pads, matching the reference's zero-padded conv
    lp = np.zeros((B, C, D, Hp, Wp), np.float32)
    lp[:, 1:] = NEG
    lp[:, :, :, 1:H + 1, 1:W + 1] = logits
    # partition-major [p = 2*d + b, class, h, w] so one DMA per chunk works
    lp = lp.transpose(2, 0, 1, 3, 4).reshape(2 * D, C, Hp, Wp).astype(F8ML)

    # one-hot masks for classes 1..3, zero pads; layout [p, cls, h, w]
    mk = np.zeros((3, B, D, Hp, Wp), np.float32)
    for i, c in enumerate((1, 2, 3)):
        mk[i, :, :, 1:H + 1, 1:W + 1] = (targets == c)
    mk = mk.transpose(2, 1, 0, 3, 4).reshape(2 * D, 3, Hp, Wp).astype(F8ML)

    td, ident = _stationaries()
    w4 = np.empty((128, 4, 2, 128), np.float32)
    w4[:, 0, 0], w4[:, 0, 1] = td, -td         # c: T_D tap
    w4[:, 1, 0], w4[:, 1, 1] = ident, -ident   # c: shift taps
    w4[:, 2, 0], w4[:, 2, 1] = td, td          # s: T_D tap
    w4[:, 3, 0], w4[:, 3, 1] = ident, ident    # s: shift taps / denominator
    w4 = w4.astype(F8ML)
    wi = ident.astype(F8ML)

    in_maps = []
    for k in range(NCORES):
        h0 = k * HS
        in_maps.append({
            "logits": np.ascontiguousarray(lp[:, :, h0:h0 + HL, :]),
            "masks": np.ascontiguousarray(mk[:, :, h0:h0 + HL, :]),
            "wts": w4,
            "wI": wi,
        })
    return in_maps


def kernel(logits, targets):
    nc = _get_nc()
    in_maps = make_in_maps(logits, targets)
    results = run_bass_kernel_spmd(nc, in_maps, core_ids=list(range(NCORES)))
    total = 0.0
    for r in results.results:
        o = np.asarray(r["out"], dtype=np.float64)
        total += o[:, 0].sum() - o[:, 1].sum()
    return np.float32(total / NTOT)
